# revision 3
# baseline (speedup 1.0000x reference)
"""TRN2 Bass kernel for nn_GQA_22436909154699 — optimized v3.

Reference math: softmax over a size-1 axis is identically 1.0, so
    out[b,l,g,h,:] = v[b,l,g,:]          (v = v-half of x @ Wkv + bkv)
The q projection (x @ Wq) never affects the output.  The kernel computes
    y = x @ Wv + bv                      (K=2048, N=256)
data-parallel over tokens across 8 NeuronCores (2048 tokens each).

Key optimizations vs the fp32 baseline:
  - bf16 operands (fp32 PSUM accumulate): fp32 moving operands stream
    the PE at 1/4 rate, bf16 at full rate; also halves x DMA traffic.
  - weight-stationary matmuls: lhsT = Wv k-tile [128k x 128n], rhs =
    x [128k x 256t] -> psum [128n x 256t].
  - the device emits only the 256 unique v-columns per token; the 8x
    heads-per-group replication is pure data movement done at unshard.
  - HWDGE DMAs on one queue complete FIFO in issue order, so the issue
    order is delivery order: wv half 0, first x block, wv half 1, bias,
    then remaining x blocks.  256-token x blocks (1 MB) keep the PE
    chasing the DMA stream with minimal first-block latency.
  - dummy warm-up matmuls on zeros run during the DMA fill so the HAM
    clock gate is at 8/8 (2.4 GHz) when real matmuls start.
"""

import os

import numpy as np

# Problem constants (hardcoded; harness runs kernel.py standalone).
B, L, E = 4, 4096, 2048
G, HPG, D = 4, 8, 64
NV = G * D  # 256 v-columns
NH = NV // 128  # 2 column halves (PE stationary is 128 wide)
NCORES = 8
TOK = B * L  # 16384 tokens
TPC = TOK // NCORES  # 2048 tokens per core
TBLK = 256  # tokens per matmul rhs
TB = TPC // TBLK  # 8 token blocks per core
KO = E // 128  # 16 contraction tiles

_CACHE: dict = {}
LAST_RESULTS = None


def _build(warmup: int):
    import concourse.bacc as bacc
    import concourse.mybir as mybir
    import concourse.tile as tile

    F32 = mybir.dt.float32
    BF16 = mybir.dt.bfloat16

    nc = bacc.Bacc(
        "TRN2", target_bir_lowering=False, debug=False, num_devices=NCORES
    )
    xt_d = nc.dram_tensor("xt", [TB, 128, KO, TBLK], BF16, kind="ExternalInput")
    wv_d = nc.dram_tensor("wv", [NH, 128, KO, 128], BF16, kind="ExternalInput")
    bias_d = nc.dram_tensor("bias", [128, NH], F32, kind="ExternalInput")
    out_d = nc.dram_tensor("out", [NH, TB, 128, TBLK], F32, kind="ExternalOutput")

    with tile.TileContext(nc) as tc:
        with (
            tc.tile_pool(name="const", bufs=1) as cpool,
            tc.tile_pool(name="xin", bufs=TB) as xpool,
            tc.tile_pool(name="obuf", bufs=4) as opool,
            tc.tile_pool(name="ps", bufs=8, space="PSUM") as ppool,
        ):
            # PE warm-up on zeros: keeps the HAM activity window busy
            # while the first DMAs land, so real matmuls start at 2.4 GHz.
            if warmup:
                zt = cpool.tile([128, 512], BF16)
                nc.vector.memset(zt[:], 0.0)
                wps = ppool.tile([128, 512], F32, tag="ps")
                for _ in range(warmup):
                    nc.tensor.matmul(
                        wps[:], lhsT=zt[:, :128], rhs=zt[:], start=True, stop=True
                    )

            # DMA issue order == FIFO delivery order.
            wvs = []
            wv0 = cpool.tile([128, KO, 128], BF16, tag="wv0")
            nc.sync.dma_start(wv0[:], wv_d[0])
            wvs.append(wv0)

            xin = []
            xt0 = xpool.tile([128, KO, TBLK], BF16, tag="xin")
            nc.sync.dma_start(xt0[:], xt_d[0])
            xin.append(xt0)

            wv1 = cpool.tile([128, KO, 128], BF16, tag="wv1")
            nc.sync.dma_start(wv1[:], wv_d[1])
            wvs.append(wv1)
            bias_sb = cpool.tile([128, NH], F32)
            nc.sync.dma_start(bias_sb[:], bias_d[:])

            for tb in range(1, TB):
                xt = xpool.tile([128, KO, TBLK], BF16, tag="xin")
                nc.sync.dma_start(xt[:], xt_d[tb])
                xin.append(xt)

            for tb in range(TB):
                for nh in range(NH):
                    ps = ppool.tile([128, TBLK], F32, tag="ps")
                    for k in range(KO):
                        nc.tensor.matmul(
                            ps[:],
                            lhsT=wvs[nh][:, k, :],
                            rhs=xin[tb][:, k, :],
                            start=(k == 0),
                            stop=(k == KO - 1),
                        )
                    ot = opool.tile([128, TBLK], F32, tag="ot")
                    nc.vector.tensor_add(
                        ot[:],
                        ps[:],
                        bias_sb[:, nh, None].to_broadcast([128, TBLK]),
                    )
                    nc.sync.dma_start(out_d[nh, tb], ot[:])
    nc.compile()
    return nc


def _get_nc():
    # 14 warm-up matmuls measured best (ends just as the first x block
    # lands; fewer lets the HAM window reset, more delays real matmuls)
    warmup = 14
    key = ("nc3", warmup)
    if key not in _CACHE:
        _CACHE[key] = _build(warmup)
    return _CACHE[key]


def _to_bf16(a):
    import ml_dtypes

    return a.astype(ml_dtypes.bfloat16)


def _prep_inputs(x, Wkv, bkv):
    x = np.asarray(x, dtype=np.float32).reshape(TOK, E)
    Wkv = np.asarray(Wkv, dtype=np.float32)
    bkv = np.asarray(bkv, dtype=np.float32)

    xb = _to_bf16(x)
    # (core, tb, t, ko, p) -> (core, tb, p, ko, t)
    xt = xb.reshape(NCORES, TB, TBLK, KO, 128).transpose(0, 1, 4, 3, 2)
    xt = np.ascontiguousarray(xt)

    # v-columns of the kv projection: Wkv reshaped (E, G, 2, D), kv index 1.
    wv = Wkv.reshape(E, G, 2, D)[:, :, 1, :].reshape(E, NV)  # (2048, 256)
    # (ko, p, nh, n) -> (nh, p, ko, n)
    wv_dev = np.ascontiguousarray(
        _to_bf16(wv).reshape(KO, 128, NH, 128).transpose(2, 1, 0, 3)
    )
    bv = bkv.reshape(G, 2, D)[:, 1, :].reshape(NV)  # (256,)
    bias_dev = np.ascontiguousarray(bv.reshape(NH, 128).T).astype(np.float32)
    return xt, wv_dev, bias_dev


def kernel(x, Wq, bq, Wkv, bkv):
    global LAST_RESULTS
    from concourse.bass_utils import run_bass_kernel_spmd

    nc = _get_nc()
    xt, wv_dev, bias_dev = _prep_inputs(x, Wkv, bkv)
    in_maps = [
        {"xt": xt[c], "wv": wv_dev, "bias": bias_dev} for c in range(NCORES)
    ]
    res = run_bass_kernel_spmd(nc, in_maps, core_ids=list(range(NCORES)))
    LAST_RESULTS = res
    # (NH, TB, 128n, TBLKt) -> (TB, t, NH, n) -> (TPC, NV)
    y = np.stack(
        [
            res.results[c]["out"].transpose(1, 3, 0, 2).reshape(TPC, NV)
            for c in range(NCORES)
        ]
    ).reshape(TOK, NV)
    out = np.broadcast_to(
        y.reshape(TOK, G, 1, D), (TOK, G, HPG, D)
    ).reshape(B, L, E)
    return np.ascontiguousarray(out).astype(np.float32)
